# revision 17
# baseline (speedup 1.0000x reference)
"""ChebGraphConv (K=3) Trainium2 kernel.

y = x@(W0-W2) - (A@x)@W1 + 2*A@((A@x)@W2) + bias

computed per (b,t) slice as:
  P0 = X@W02 ; P1 = X@W1 ; P2' = X@(2*W2)   (one 192-wide bf16 matmul per
                                             node block, X bf16, no hi/lo)
  Q' = A@P2' ; M = P1 - Q' ; S = A@M ; y = P0 - S (+bias)

A is stored in fp8e5m2 at natural scale (entries ~U(0,1/2048) sit in e5m2's
normal range), so the two spmms run as fp8 DoubleRow matmuls with NO descale
op: each spmm k-block needs exactly one DVE subtract to evacuate PSUM.
P1/P2'/M are e5m2 (they only reach y through the small A@(...) terms,
|A@v| ~ 0.02*|v|); P0 is bf16 (dominant term). Measured numpy rel err ~5e-3
vs the 2e-2 gate.

Projections write 4 node blocks per PSUM tile (2 banks, 256-col spacing) so
evacuation is 2 ops per 4 blocks (one fp8 p12 copy, one bf16 P0 copy),
alternated between DVE and Act to balance engine load.

Data parallel over B: core b handles x[b] (T=12 slices), 2 groups of 6
slices; group 1's projections interleave into group 0's spmm phases. y is
written [N, T, C] per core (contiguous DMA) and transposed on host.
"""

import numpy as np
import ml_dtypes

import concourse.bacc as bacc
import concourse.mybir as mybir
import concourse.tile as tile
from concourse import bass_utils

BF16 = ml_dtypes.bfloat16
FP8E5 = ml_dtypes.float8_e5m2

B, T, N, C = 8, 12, 2048, 64
NB = N // 128          # 16 node blocks
NMT = NB // 2          # 8 DoubleRow contraction tiles (256 nodes each)
G = 6                  # slices per group
GW = G * C             # 384

_NC_CACHE = {}


def _build_nc(repeat=None, with_bias=False):
    """repeat=None: single-shot kernel (graded path). repeat=R: wraps the
    whole body in a hardware For loop running it R times (benchmarking)."""
    key = ("nc", repeat, with_bias)
    if key in _NC_CACHE:
        return _NC_CACHE[key]
    f32 = mybir.dt.float32
    bf16 = mybir.dt.bfloat16
    fp8 = mybir.dt.float8e5

    nc = bacc.Bacc("TRN2", target_bir_lowering=False, debug=False,
                   enable_asserts=False, num_devices=8)

    at_d = nc.dram_tensor("at8", [NB, 128, NMT, 256], fp8,
                          kind="ExternalInput")
    xs_d = nc.dram_tensor("xs", [T // 2, 128, N], bf16, kind="ExternalInput")
    wa_d = nc.dram_tensor("wa", [128, 3 * C], bf16, kind="ExternalInput")
    bias_d = nc.dram_tensor("biasb", [128, C], f32, kind="ExternalInput")
    y_d = nc.dram_tensor("y", [N, T, C], f32, kind="ExternalOutput")

    with tile.TileContext(nc) as tc:
        with (
            tc.tile_pool(name="const", bufs=1) as constp,
            tc.tile_pool(name="atp", bufs=2) as atp,
            tc.tile_pool(name="xsp", bufs=2) as xsp,
            tc.tile_pool(name="p12p", bufs=2) as p12p,
            tc.tile_pool(name="p0p", bufs=2) as p0p,
            tc.tile_pool(name="mp", bufs=2) as mp,
            tc.tile_pool(name="ystage", bufs=3) as ystage,
            tc.tile_pool(name="pps", bufs=2, space="PSUM") as pps,
            tc.tile_pool(name="sps", bufs=4, space="PSUM") as sps,
        ):
            def emit_body():
                _emit(nc, constp, atp, xsp, p12p, p0p, mp, ystage, pps, sps,
                      at_d, xs_d, wa_d, bias_d, y_d, with_bias)

            if repeat is None:
                emit_body()
            else:
                with tc.For_i(0, repeat, 1):
                    emit_body()

    nc.compile()
    _NC_CACHE[key] = nc
    return nc


def _emit(nc, constp, atp, xsp, p12p, p0p, mp, ystage, pps, sps,
          at_d, xs_d, wa_d, bias_d, y_d, with_bias):
    f32 = mybir.dt.float32
    bf16 = mybir.dt.bfloat16
    fp8 = mybir.dt.float8e5

    wa_t = constp.tile([128, 3 * C], bf16, tag="wa")
    bias_t = constp.tile([128, C], f32, tag="bias")

    # one xs tile per slice-pair: [128, N] with partitions = 2 slices x 64 c,
    # and one A^T tile per output node block k (holding all 8 DoubleRow
    # stationaries for that block). DMAs all serialize on the shared DMA
    # engines, so the issue ORDER is chosen to match consumption order:
    # proj pair p needs xs_t[p], spmm chain k needs only at_t[k].
    xs_t = [xsp.tile([128, N], bf16, tag=f"xs{p}", name=f"xs{p}")
            for p in range(T // 2)]
    at_t = [atp.tile([128, NMT, 256], fp8, tag=f"at{k}", name=f"at{k}")
            for k in range(NB)]
    nc.sync.dma_start(xs_t[0][:], xs_d[0, :, :])
    nc.sync.dma_start(wa_t[:], wa_d[:, :])
    nc.sync.dma_start(bias_t[:], bias_d[:, :])
    nc.sync.dma_start(xs_t[1][:], xs_d[1, :, :])
    nc.sync.dma_start(xs_t[2][:], xs_d[2, :, :])
    for k in range(NB):
        nc.sync.dma_start(at_t[k][:], at_d[k, :, :, :])
    for p in range(3, T // 2):
        nc.sync.dma_start(xs_t[p][:], xs_d[p, :, :])

    def xstat(g, idx, k):
        """Stationary [64, 128] for slice idx of group g, node block k."""
        t = g * G + idx
        return xs_t[t // 2][(t % 2) * C:(t % 2 + 1) * C, k * 128:(k + 1) * 128]

    def proj_step(g, p12, p0, idx, q):
        """Projection for slice idx of group g, node blocks 4q..4q+3.
        pp columns per block j (at 256-col spacing): [P1|P2'|P0|pad]."""
        cs = slice(idx * C, (idx + 1) * C)
        pp = pps.tile([128, 1024], f32, tag="pp", name="pp")
        for j in range(4):
            k = 4 * q + j
            h = ((g * G + idx) % 2) * C
            nc.tensor.matmul(pp[:, j * 256:j * 256 + 192],
                             xstat(g, idx, k),
                             wa_t[h:h + C, :], start=True, stop=True)
        pv = pp.rearrange("p (j pl c) -> p j pl c", j=4, pl=4, c=C)
        s = idx * 4 + q
        # p12 copy: [128, 4(k), 2(pl), 64] fp8; P0 copy: [128, 4(k), 64] bf16
        p12_dst = p12[q][:, :, :, cs]
        p0_dst = p0[q][:, :, cs]
        if g == 1:
            # interleaved steps: p12 on Act, P0 on DVE (DVE also carries the
            # spmm subtracts; this split evens both at ~1 op per k-window)
            nc.scalar.copy(p12_dst, pv[:, :, 0:2, :])
            nc.vector.tensor_copy(p0_dst, pv[:, :, 2, :])
        elif s % 2 == 0:
            nc.vector.tensor_copy(p12_dst, pv[:, :, 0:2, :])
            nc.scalar.copy(p0_dst, pv[:, :, 2, :])
        else:
            nc.scalar.copy(p12_dst, pv[:, :, 0:2, :])
            nc.vector.tensor_copy(p0_dst, pv[:, :, 2, :])

    def dr_chain(sp, k, moving):
        """One fp8 DoubleRow accumulation chain: sp += A^T[kblk] @ moving.
        moving: [128, 2, GW] fp8 slices of a [128, NB, GW]-shaped view."""
        for mt in range(NMT):
            nc.tensor.matmul(sp[:], at_t[k][:, mt, :],
                             moving(mt),
                             start=(mt == 0), stop=(mt == NMT - 1),
                             perf_mode=mybir.MatmulPerfMode.DoubleRowSwInterleave)

    def interleave_emit(interleave, k, total_k):
        if not interleave:
            return
        nchunk = len(interleave)
        c0 = k * nchunk // total_k
        c1 = (k + 1) * nchunk // total_k
        for thunk in interleave[c0:c1]:
            thunk()

    def spmm2(p12, m, interleave=None, ilv_k0=0):
        """Q' = A@P2' ; M = P1 - Q' (one DVE subtract per k-block)."""
        for k in range(NB):
            if k >= ilv_k0:
                interleave_emit(interleave, k - ilv_k0, NB - ilv_k0)
            sp = sps.tile([128, GW], f32, tag="sp", name="sp")
            dr_chain(sp, k, lambda mt: p12[mt // 2][:, 2 * (mt % 2):
                                                    2 * (mt % 2) + 2, 1, :])
            nc.vector.tensor_tensor(m[:, k, :], p12[k // 4][:, k % 4, 0, :],
                                    sp[:], op=mybir.AluOpType.subtract)

    def spmm3(m, p0, s0, interleave=None):
        """S = A@M ; y = P0 - S (+bias)."""
        for k in range(NB):
            interleave_emit(interleave, k, NB)
            sp = sps.tile([128, GW], f32, tag="sp", name="sp")
            dr_chain(sp, k, lambda mt: m[:, 2 * mt:2 * mt + 2, :])
            yt = ystage.tile([128, GW], f32, tag="y", name="yt")
            nc.vector.tensor_sub(yt[:], p0[k // 4][:, k % 4, :], sp[:])
            if with_bias:
                for idx in range(G):
                    ysl = yt[:, idx * C:(idx + 1) * C]
                    nc.vector.tensor_tensor(ysl, ysl, bias_t[:],
                                            op=mybir.AluOpType.add)
            nc.sync.dma_start(y_d[k * 128:(k + 1) * 128, s0:s0 + G, :], yt[:])

    # per-q tiles (4 node blocks each) so spmm chains can start as soon as
    # the first q's projections land; p12 layout [128, 4(k), 2(pl), GW]
    def group_tiles(g):
        p12 = [p12p.tile([128, 4, 2, GW], fp8, tag=f"p12q{q}",
                         name=f"p12_{g}q{q}") for q in range(4)]
        p0 = [p0p.tile([128, 4, GW], bf16, tag=f"p0q{q}",
                       name=f"p0_{g}q{q}") for q in range(4)]
        m = mp.tile([128, NB, GW], fp8, tag="m", name=f"m_{g}")
        return p12, p0, m

    p12_0, p0_0, m_0 = group_tiles(0)
    for q in range(NB // 4):
        for idx in range(G):
            proj_step(0, p12_0, p0_0, idx, q)

    # group 1 proj rides inside group 0's spmm phases (its PSUM evacuation
    # hides under the spmm chains); needs p12/p0 bufs=2
    p12_1, p0_1, m_1 = group_tiles(1)
    proj1 = [(lambda i=idx, qq=q: proj_step(1, p12_1, p0_1, i, qq))
             for q in range(NB // 4) for idx in range(G)]
    spmm2(p12_0, m_0, interleave=proj1[:8], ilv_k0=8)
    spmm3(m_0, p0_0, 0, interleave=proj1[8:])

    spmm2(p12_1, m_1)
    spmm3(m_1, p0_1, G)


def _prep_inputs(x, A_norm, weight, bias):
    """Host-side shard + layout prep. Returns per-core input maps."""
    x = np.asarray(x, dtype=np.float32)
    A_norm = np.asarray(A_norm, dtype=np.float32)
    weight = np.asarray(weight, dtype=np.float32)
    bias = np.asarray(bias, dtype=np.float32)

    # per-k-block DoubleRowSwInterleave A^T pack: for each (kb, mt) the
    # 256-wide stationary holds [A127,B127,A126,B126,...,A0,B0] per
    # partition kp, where A/B = planes i=0/1 and column n' runs reversed:
    # at8[kb, kp, mt, 2*(127-n')+i] = A[kb*128+n', mt*256 + i*128 + kp]
    A2 = A_norm.reshape(NB, 128, NMT, 2, 128)        # [kb, n', mt, i, kp]
    at8 = A2.transpose(0, 4, 2, 1, 3)[:, :, :, ::-1, :]  # [kb, kp, mt, n'r, i]
    at8_host = np.ascontiguousarray(at8.reshape(NB, 128, NMT, 256)).astype(FP8E5)

    W0, W1, W2 = weight[0], weight[1], weight[2]
    wa_host = np.zeros((128, 3 * C), dtype=BF16)
    for h in (0, C):
        wa_host[h:h + C, 0:C] = W1.astype(BF16)
        wa_host[h:h + C, C:2 * C] = (2.0 * W2).astype(BF16)
        wa_host[h:h + C, 2 * C:3 * C] = (W0 - W2).astype(BF16)

    bias_host = np.ascontiguousarray(np.broadcast_to(bias, (128, C)),
                                     dtype=np.float32)

    in_maps = []
    for b in range(B):
        xt = x[b].transpose(0, 2, 1)                 # [T, C, N]
        xt = xt.reshape(T // 2, 128, N)              # pair slices on partitions
        in_maps.append({
            "at8": at8_host,
            "xs": np.ascontiguousarray(xt).astype(BF16),
            "wa": wa_host,
            "biasb": bias_host,
        })
    return in_maps


def kernel(x, A_norm, weight, bias):
    with_bias = bool(np.any(np.asarray(bias)))
    nc = _build_nc(with_bias=with_bias)
    in_maps = _prep_inputs(x, A_norm, weight, bias)
    last_err = None
    for attempt in range(3):
        try:
            res = bass_utils.run_bass_kernel_spmd(nc, in_maps,
                                                  core_ids=list(range(8)))
            break
        except Exception as e:  # transient NRT_EXEC_UNIT_UNRECOVERABLE etc.
            last_err = e
            import time
            time.sleep(2.0 * (attempt + 1))
    else:
        raise last_err
    # per-core y is [N, T, C]; full output is [B, T, N, C]
    out = np.stack([res.results[b]["y"].transpose(1, 0, 2) for b in range(B)],
                   axis=0)
    return np.ascontiguousarray(out, dtype=np.float32)
